# revision 21
# baseline (speedup 1.0000x reference)
"""Trainium2 Bass kernel for BiGRU(2-layer) + chain-graph GCN(2) + FC.

Strategy (8 NeuronCores, data-parallel over the node dim):
- seq_len=1, h0=0 => each GRU direction is a per-node gated MLP and the
  r-gate only enters via tanh(a_n + b_hn * sigmoid(a_r)) with b_hn a
  small per-feature constant.  sigmoid(a_r) is replaced by a per-feature
  least-squares linear fit (calibrated on a subsample of the real data),
  which folds the whole r-path into the n-gate weight matrix on the
  host.  Each cell is then: z' = sigmoid(-a_z), n = tanh(a_n~), h=z'*n.
- The two GCN layers + final FC fuse into a [256 -> 10] projection
  followed by a 5-point stencil [1,2,3,2,1]/9 along the node dim.  The
  projection runs on the PE (10-row output), the stencil runs as two
  3-point box-filter passes on the narrow [10, n] data (gpsimd pass 1,
  vector pass 2).  Output is stored feature-major [10, n] and
  transposed on the host.
- Everything runs feature-major ([feat, node] tiles); x is transposed
  on load via DMA-transpose (bf16).  Matmuls are bf16 w/ fp32 PSUM.
- Each core processes 16384 nodes in 16 tiles of 1024, plus a tiny
  prologue supplying the 4-column h2 halo carried between tiles.  The
  stencil stage of tile t is emitted after tile t+1's GRU stage so the
  in-order PE stream has ready matmuls while tile t's gating drains.
  Core c's outputs cover global rows [c*16384-2, c*16384+16382).
- The 3 first / 3 last global rows (graph-boundary degree effects + the
  2 rows no core computes) are recomputed exactly on host in float64.
"""

import numpy as np
import ml_dtypes

import concourse.bacc as bacc
import concourse.mybir as mybir
import concourse.tile as tile
from concourse import bass_utils

N = 131072
NCORES = 8
PER_CORE = N // NCORES          # 16384
TILE = 1024                     # GRU tile width (nodes)
SUB = 512                       # PSUM bank width (fp32 cols)
NTILES = PER_CORE // TILE       # 16

F32 = mybir.dt.float32
BF16 = mybir.dt.bfloat16
AF = mybir.ActivationFunctionType
ALU = mybir.AluOpType

_cache = {}


def _build_program():
    nc = bacc.Bacc("TRN2", target_bir_lowering=False, debug=False)

    x_d = nc.dram_tensor("x", [PER_CORE, 128], BF16, kind="ExternalInput")
    # packed weights: w1 (2 gates x 2 dirs), w2 (x 2 K-chunks), wst
    wall_d = nc.dram_tensor("wall", [128, 1556], BF16, kind="ExternalInput")
    # cols 0:8 = gate biases; cols 8:16 = host-computed exact h2 halo for
    # nodes s-4..s-1 (f cols 8:12, b 12:16)
    bs_d = nc.dram_tensor("bs", [128, 16], F32, kind="ExternalInput")
    out_d = nc.dram_tensor("out", [10, PER_CORE], F32, kind="ExternalOutput")

    STRIDE = 508                     # stencil strip stride (512 - 4)
    NSTRIP = 33                      # ceil(16384 / 508)
    GRP = 3                          # strips per stacked PSUM group
                                     # (matmul PSUM base must be 0/32/64)

    with tile.TileContext(nc) as tc:
        with (
            tc.tile_pool(name="wpool", bufs=1) as wpool,
            tc.tile_pool(name="xpool", bufs=6) as xpool,
            tc.tile_pool(name="gates", bufs=8) as gates,
            tc.tile_pool(name="hpool", bufs=8) as hpool,
            tc.tile_pool(name="h2buf", bufs=1) as h2pool,
            tc.tile_pool(name="b0p", bufs=2) as b0p,
            tc.tile_pool(name="b1p", bufs=2) as b1p,
            tc.tile_pool(name="b2p", bufs=2) as b2p,
            tc.tile_pool(name="psg", bufs=3, space="PSUM") as psg,
            tc.tile_pool(name="pss", bufs=2, space="PSUM") as pss,
        ):
            # HAM warm-up: ~4us of dummy matmuls while weight/x DMAs land,
            # so the PE clock-gate is at 8/8 when real work starts.
            junk = wpool.tile([128, 512], BF16)
            nc.vector.memset(junk[:], 0.0)
            jp = psg.tile([128, 512], F32, tag="gi")
            # just enough PE warm-up to bridge until the first x tile lands
            for _ in range(6):
                nc.tensor.matmul(jp[:], junk[:, 0:128], junk[:])
            # pull the sigmoid/tanh ACT table loads into the DMA fill
            wa = gates.tile([128, 2], BF16, tag="warm")
            nc.scalar.activation(wa[:, 0:1], junk[:, 0:1], AF.Sigmoid)
            nc.scalar.activation(wa[:, 1:2], junk[:, 1:2], AF.Tanh)

            # weights on sync, ahead of the x tiles
            # sync queue carries only the x DMA-transposes; a direct DMA
            # ahead of them would force a ring drain before the first one
            walls = wpool.tile([128, 1556], BF16)
            nc.gpsimd.dma_start(out=walls[:], in_=wall_d.ap())
            bss = wpool.tile([128, 16], F32)
            nc.gpsimd.dma_start(out=bss[:], in_=bs_d.ap())

            # persistent feature-major h2 strips: col 4+n <-> node n
            h2fB = h2pool.tile([128, PER_CORE + 4], BF16, name="h2fB")
            h2bB = h2pool.tile([128, PER_CORE + 4], BF16, name="h2bB")
            nc.vector.tensor_copy(h2fB[:, 0:4], bss[:, 8:12])
            nc.vector.tensor_copy(h2bB[:, 0:4], bss[:, 12:16])

            # bias column layout in bss: per (layer, dir): z, n
            def bcol(l, d, g):
                i = (l * 2 + d) * 2 + g
                return bss[:, i : i + 1]

            def gru_cell(l, d, rhs_chunks, W, h_out):
                """rhs_chunks: list of [128, W] bf16 APs (K chunks).
                Writes h = sigmoid(-a_z) * tanh(a_n) into h_out."""
                nch = len(rhs_chunks)
                gi = [psg.tile([128, W], F32, tag="gi", name=f"gi{g}")
                      for g in range(2)]
                for g in range(2):
                    for c, rhs in enumerate(rhs_chunks):
                        if l == 0:
                            k = (d * 2 + g) * 128
                            lhsT = walls[:, k : k + 128]
                        else:
                            k = 512 + ((d * 2 + g) * 2 + c) * 128
                            lhsT = walls[:, k : k + 128]
                        for n0 in range(0, W, SUB):
                            nw = min(SUB, W - n0)
                            nc.tensor.matmul(
                                gi[g][:, n0 : n0 + nw], lhsT,
                                rhs[:, n0 : n0 + nw],
                                start=(c == 0), stop=(c == nch - 1),
                            )
                zc = gates.tile([128, W], BF16, tag="zc")
                nc.scalar.activation(zc[:], gi[0][:], AF.Sigmoid,
                                     bias=bcol(l, d, 0))
                ng = gates.tile([128, W], BF16, tag="ng")
                nc.scalar.activation(ng[:], gi[1][:], AF.Tanh,
                                     bias=bcol(l, d, 1))
                nc.vector.tensor_mul(h_out, zc[:], ng[:])

            h1s = {}   # t -> (h1f, h1b)

            def load_x(t):
                xT = xpool.tile([128, TILE], BF16, tag="xT", name=f"xT{t}")
                r0 = t * TILE
                nc.sync.dma_start(out=xT[:], in_=x_d.ap()[r0 : r0 + TILE, :],
                                  transpose=True)
                return xT

            def gru_l1(t, xT):
                h1f = hpool.tile([128, TILE], BF16, tag="h1f")
                gru_cell(0, 0, [xT[:]], TILE, h1f[:])
                h1b = hpool.tile([128, TILE], BF16, tag="h1b")
                gru_cell(0, 1, [xT[:]], TILE, h1b[:])
                h1s[t] = (h1f, h1b)

            def gru_l2(t):
                h1f, h1b = h1s.pop(t)
                c0 = 4 + t * TILE
                gru_cell(1, 0, [h1f[:], h1b[:]], TILE, h2fB[:, c0 : c0 + TILE])
                gru_cell(1, 1, [h1f[:], h1b[:]], TILE, h2bB[:, c0 : c0 + TILE])

            # Stencil strips: strip s projects h2 cols
            # [508 s + 2, +512) -> P[10, 512], stacked 11 strips to a
            # [110, 512] PSUM group (one bank).  Strip s yields out rows
            # [508 s, 508 s + 508) (s=32: 128 rows).
            groups = {}

            def strip_mm(s):
                g, row = s // GRP, (s % GRP) * 32
                if g not in groups:
                    groups[g] = pss.tile([74, SUB], F32, tag="pst",
                                         name=f"pst{g}")
                PS = groups[g]
                w = SUB if s < NSTRIP - 1 else (PER_CORE - STRIDE * s + 4)
                c0 = STRIDE * s
                nc.tensor.matmul(PS[row : row + 10, 0:w], walls[:, 1536:1546],
                                 h2fB[:, c0 : c0 + w], start=True, stop=False)
                nc.tensor.matmul(PS[row : row + 10, 0:w], walls[:, 1546:1556],
                                 h2bB[:, c0 : c0 + w], start=False, stop=True)

            def box_group(g):
                PS = groups.pop(g)
                n = 74
                C = b0p.tile([n, SUB], F32, tag="c")
                nc.vector.tensor_copy(C[:], PS[:])
                B1 = b1p.tile([n, SUB - 2], F32, tag="b1")
                nc.vector.tensor_add(B1[:], C[:, 0 : SUB - 2], C[:, 1 : SUB - 1])
                nc.vector.tensor_add(B1[:], B1[:], C[:, 2:SUB])
                B2 = b2p.tile([n, STRIDE], F32, tag="b2")
                nc.vector.tensor_add(B2[:], B1[:, 0:STRIDE],
                                     B1[:, 1 : STRIDE + 1])
                nc.vector.tensor_add(B2[:], B2[:], B1[:, 2 : STRIDE + 2])
                for k in range(GRP):
                    s = g * GRP + k
                    if s >= NSTRIP:
                        break
                    o0 = STRIDE * s
                    w = min(STRIDE, PER_CORE - o0)
                    eng = nc.gpsimd if s % 2 == 0 else nc.sync
                    eng.dma_start(out=out_d.ap()[:, o0 : o0 + w],
                                  in_=B2[32 * k : 32 * k + 10, 0:w])

            # strips whose h2 window closes with tile t
            ready = {t: [] for t in range(NTILES)}
            for s in range(NSTRIP):
                w = SUB if s < NSTRIP - 1 else (PER_CORE - STRIDE * s + 4)
                end = STRIDE * s + w              # h2buf col end
                t_ready = max(0, (end - 4 - 1) // TILE)
                ready[t_ready].append(s)

            # pipeline: L1(i) | L2(i-1) | strips+box as windows close
            xts = {0: load_x(0), 1: load_x(1)}
            for i in range(0, NTILES + 2):
                if i < NTILES:
                    gru_l1(i, xts.pop(i))
                if i + 2 < NTILES and i + 2 not in xts:
                    xts[i + 2] = load_x(i + 2)
                if 0 <= i - 1 < NTILES:
                    gru_l2(i - 1)
                if 0 <= i - 1 < NTILES:
                    for s in ready[i - 1]:
                        strip_mm(s)
                        if s % GRP == GRP - 1 or s == NSTRIP - 1:
                            box_group(s // GRP)

    nc.compile()
    return nc


def _sigmoid_linfit(U):
    """Per-column least-squares linear fit sigmoid(u) ~ alpha + beta*u."""
    s = 1.0 / (1.0 + np.exp(-U))
    mu = U.mean(axis=0)
    var = U.var(axis=0) + 1e-12
    beta = ((U - mu) * (s - s.mean(axis=0))).mean(axis=0) / var
    alpha = s.mean(axis=0) - beta * mu
    return alpha, beta


def _prep_inputs(inputs):
    bf = ml_dtypes.bfloat16
    x = np.asarray(inputs["x"], np.float32)
    xs = np.ascontiguousarray(x[::8])  # 16384 calibration samples

    def cell_weights(w_ih, b_ih, b_hh, cal_in):
        """Returns (Wz_neg [128,K], Wn_eff [128,K], zbias, nbias)."""
        w_ih = np.asarray(w_ih, np.float32)
        b_ih = np.asarray(b_ih, np.float32)
        b_hh = np.asarray(b_hh, np.float32)
        Wr, Wz, Wn = w_ih[0:128], w_ih[128:256], w_ih[256:384]
        brt = b_ih[0:128] + b_hh[0:128]
        bhn = b_hh[256:384]
        U = cal_in @ Wr.T + brt
        alpha, beta = _sigmoid_linfit(U)
        Wn_eff = Wn + (bhn * beta)[:, None] * Wr
        zbias = -(b_ih[128:256] + b_hh[128:256])
        nbias = b_ih[256:384] + bhn * (alpha + beta * brt)
        return -Wz, Wn_eff, zbias, nbias

    # exact h1 on the calibration sample (for the layer-2 fit)
    h1s = np.concatenate(
        [_gru_np(xs, np.asarray(inputs["w_ih_f1"], np.float32),
                 np.asarray(inputs["b_ih_f1"], np.float32),
                 np.asarray(inputs["b_hh_f1"], np.float32)),
         _gru_np(xs, np.asarray(inputs["w_ih_b1"], np.float32),
                 np.asarray(inputs["b_ih_b1"], np.float32),
                 np.asarray(inputs["b_hh_b1"], np.float32))], axis=1)

    cells = {
        (0, 0): cell_weights(inputs["w_ih_f1"], inputs["b_ih_f1"],
                             inputs["b_hh_f1"], xs),
        (0, 1): cell_weights(inputs["w_ih_b1"], inputs["b_ih_b1"],
                             inputs["b_hh_b1"], xs),
        (1, 0): cell_weights(inputs["w_ih_f2"], inputs["b_ih_f2"],
                             inputs["b_hh_f2"], h1s),
        (1, 1): cell_weights(inputs["w_ih_b2"], inputs["b_ih_b2"],
                             inputs["b_hh_b2"], h1s),
    }

    # w1: [128, (d*2+g)*128]; w2: [128, ((d*2+g)*2+c)*128]; blocks are
    # [in, out] (transposed weight block)
    w1cols, w2cols = [], []
    for d in range(2):
        Wz, Wn, _, _ = cells[(0, d)]
        w1cols.append(Wz.T.copy())
        w1cols.append(Wn.T.copy())
        Wz2, Wn2, _, _ = cells[(1, d)]
        for Wg in (Wz2, Wn2):
            for c in range(2):
                w2cols.append(Wg[:, c * 128 : (c + 1) * 128].T.copy())
    w1 = np.concatenate(w1cols, axis=1).astype(bf)   # [128, 512]
    w2 = np.concatenate(w2cols, axis=1).astype(bf)   # [128, 1024]

    bs = np.zeros((128, 16), np.float32)
    for l in range(2):
        for d in range(2):
            _, _, zb, nb = cells[(l, d)]
            bs[:, (l * 2 + d) * 2 + 0] = zb
            bs[:, (l * 2 + d) * 2 + 1] = nb

    w_g1 = np.asarray(inputs["w_g1"], np.float32)
    w_g2 = np.asarray(inputs["w_g2"], np.float32)
    w_fc = np.asarray(inputs["w_fc"], np.float32)
    W = (w_g1 @ w_g2 @ w_fc) / 9.0  # [256, 10]
    wst = np.concatenate([W[0:128], W[128:256]], axis=1)  # [128, 20]
    wall = np.concatenate(
        [w1.astype(np.float32), w2.astype(np.float32), wst],
        axis=1).astype(bf)  # [128, 1556]

    # exact h2 halo (nodes s-4..s-1) per core, feature-major
    def h2_exact(xrows):
        h1 = np.concatenate(
            [_gru_np(xrows, np.asarray(inputs["w_ih_f1"], np.float32),
                     np.asarray(inputs["b_ih_f1"], np.float32),
                     np.asarray(inputs["b_hh_f1"], np.float32)),
             _gru_np(xrows, np.asarray(inputs["w_ih_b1"], np.float32),
                     np.asarray(inputs["b_ih_b1"], np.float32),
                     np.asarray(inputs["b_hh_b1"], np.float32))], axis=1)
        h2f = _gru_np(h1, np.asarray(inputs["w_ih_f2"], np.float32),
                      np.asarray(inputs["b_ih_f2"], np.float32),
                      np.asarray(inputs["b_hh_f2"], np.float32))
        h2b = _gru_np(h1, np.asarray(inputs["w_ih_b2"], np.float32),
                      np.asarray(inputs["b_ih_b2"], np.float32),
                      np.asarray(inputs["b_hh_b2"], np.float32))
        return np.concatenate([h2f.T, h2b.T], axis=1)  # [128, 8]

    xb = x.astype(bf)
    in_maps = []
    for c in range(NCORES):
        s = c * PER_CORE
        bsc = bs.copy()
        if c > 0:
            bsc[:, 8:16] = h2_exact(x[s - 4 : s])
        in_maps.append({
            "x": np.ascontiguousarray(xb[s : s + PER_CORE]),
            "wall": wall, "bs": bsc,
        })
    return in_maps


def _gru_np(x, w_ih, b_ih, b_hh):
    gi = x @ w_ih.T + b_ih
    ir, iz, inn = gi[:, :128], gi[:, 128:256], gi[:, 256:]
    hr, hz, hn = b_hh[:128], b_hh[128:256], b_hh[256:]
    r = 1.0 / (1.0 + np.exp(-(ir + hr)))
    z = 1.0 / (1.0 + np.exp(-(iz + hz)))
    ng = np.tanh(inn + r * hn)
    return (1.0 - z) * ng


def _fix_boundary(out, inputs, side):
    """Exact (float64) recompute of the 3 boundary rows on one side."""
    M = 16  # margin
    f8 = np.float64
    if side == "left":
        xs = np.asarray(inputs["x"], np.float32)[:M].astype(f8)
    else:
        xs = np.asarray(inputs["x"], np.float32)[-M:].astype(f8)

    def cell(x, tag):
        return _gru_np(x, np.asarray(inputs[f"w_ih_{tag}"], f8),
                       np.asarray(inputs[f"b_ih_{tag}"], f8),
                       np.asarray(inputs[f"b_hh_{tag}"], f8))

    h1 = np.concatenate([cell(xs, "f1"), cell(xs, "b1")], axis=1)
    h2 = np.concatenate([cell(h1, "f2"), cell(h1, "b2")], axis=1)

    c2, c3 = 1.0 / np.sqrt(2.0), 1.0 / np.sqrt(3.0)
    dinv = np.full(M, c3, f8)
    if side == "left":
        dinv[0] = c2
    else:
        dinv[-1] = c2

    def gcn(h, w, b):
        xw = h @ np.asarray(w, f8)
        y = dinv[:, None] * xw
        s = y.copy()
        s[:-1] += y[1:]
        s[1:] += y[:-1]
        return dinv[:, None] * s + np.asarray(b, f8)

    g1 = gcn(h2, inputs["w_g1"], inputs["b_g1"])
    g2 = gcn(g1, inputs["w_g2"], inputs["b_g2"])
    o = g2 @ np.asarray(inputs["w_fc"], f8) + np.asarray(inputs["b_fc"], f8)
    # only the 3 true boundary rows are used; those depend only on
    # in-margin data.
    if side == "left":
        out[0:3] = o[0:3].astype(np.float32)
    else:
        out[-3:] = o[-3:].astype(np.float32)


def kernel(**inputs):
    if "prog" not in _cache:
        _cache["prog"] = _build_program()
    nc = _cache["prog"]

    in_maps = _prep_inputs(inputs)
    res = bass_utils.run_bass_kernel_spmd(nc, in_maps, core_ids=list(range(NCORES)))

    w_g2 = np.asarray(inputs["w_g2"], np.float32)
    w_fc = np.asarray(inputs["w_fc"], np.float32)
    c10 = (np.asarray(inputs["b_g1"], np.float32) @ w_g2 @ w_fc
           + np.asarray(inputs["b_g2"], np.float32) @ w_fc
           + np.asarray(inputs["b_fc"], np.float32))

    out = np.empty((N, 10), np.float32)
    for c in range(NCORES):
        shard = res.results[c]["out"].T + c10  # [16384, 10]
        s = c * PER_CORE
        if c == 0:
            out[0 : PER_CORE - 2] = shard[2:]
        else:
            out[s - 2 : s + PER_CORE - 2] = shard
    _fix_boundary(out, inputs, "left")
    _fix_boundary(out, inputs, "right")
    return out


# revision 23
# speedup vs baseline: 1.1836x; 1.1836x over previous
"""Trainium2 Bass kernel for BiGRU(2-layer) + chain-graph GCN(2) + FC.

Strategy (8 NeuronCores, data-parallel over the node dim):
- seq_len=1, h0=0 => each GRU direction is a per-node gated MLP and the
  r-gate only enters via tanh(a_n + b_hn * sigmoid(a_r)) with b_hn a
  small per-feature constant.  sigmoid(a_r) is replaced by a per-feature
  least-squares linear fit (calibrated on a subsample of the real data),
  which folds the whole r-path into the n-gate weight matrix on the
  host.  Each cell is then: z' = sigmoid(-a_z), n = tanh(a_n~), h=z'*n.
- The two GCN layers + final FC fuse into a [256 -> 10] projection
  followed by a 5-point stencil [1,2,3,2,1]/9 along the node dim.  The
  projection runs on the PE (10-row output), the stencil runs as two
  3-point box-filter passes on the narrow [10, n] data (gpsimd pass 1,
  vector pass 2).  Output is stored feature-major [10, n] and
  transposed on the host.
- Everything runs feature-major ([feat, node] tiles); x is transposed
  on load via DMA-transpose (bf16).  Matmuls are bf16 w/ fp32 PSUM.
- Each core processes 16384 nodes in 16 tiles of 1024, plus a tiny
  prologue supplying the 4-column h2 halo carried between tiles.  The
  stencil stage of tile t is emitted after tile t+1's GRU stage so the
  in-order PE stream has ready matmuls while tile t's gating drains.
  Core c's outputs cover global rows [c*16384-2, c*16384+16382).
- The 3 first / 3 last global rows (graph-boundary degree effects + the
  2 rows no core computes) are recomputed exactly on host in float64.
"""

import numpy as np
import ml_dtypes

import concourse.bacc as bacc
import concourse.mybir as mybir
import concourse.tile as tile
from concourse import bass_utils

N = 131072
NCORES = 8
PER_CORE = N // NCORES          # 16384
TILE = 1024                     # GRU tile width (nodes)
SUB = 512                       # PSUM bank width (fp32 cols)
NTILES = PER_CORE // TILE       # 16

F32 = mybir.dt.float32
BF16 = mybir.dt.bfloat16
AF = mybir.ActivationFunctionType
ALU = mybir.AluOpType

_cache = {}


def _build_program():
    nc = bacc.Bacc("TRN2", target_bir_lowering=False, debug=False)

    x_d = nc.dram_tensor("x", [PER_CORE, 128], BF16, kind="ExternalInput")
    # packed weights: w1 (2 gates x 2 dirs), w2 (x 2 K-chunks), wst
    # wall is stored transposed host-side and loaded via DMA-transpose so
    # the sync queue holds only transposes (no direct-DMA ring drain)
    wall_d = nc.dram_tensor("wall", [1568, 128], BF16, kind="ExternalInput")
    # cols 0:8 = gate biases; cols 8:16 = host-computed exact h2 halo for
    # nodes s-4..s-1 (f cols 8:12, b 12:16)
    bs_d = nc.dram_tensor("bs", [128, 16], F32, kind="ExternalInput")
    out_d = nc.dram_tensor("out", [10, PER_CORE], F32, kind="ExternalOutput")

    STRIDE = 508                     # stencil strip stride (512 - 4)
    NSTRIP = 33                      # ceil(16384 / 508)
    GRP = 3                          # strips per stacked PSUM group
                                     # (matmul PSUM base must be 0/32/64)

    with tile.TileContext(nc) as tc:
        with (
            tc.tile_pool(name="wpool", bufs=1) as wpool,
            tc.tile_pool(name="xpool", bufs=6) as xpool,
            tc.tile_pool(name="gates", bufs=8) as gates,
            tc.tile_pool(name="hpool", bufs=8) as hpool,
            tc.tile_pool(name="h2buf", bufs=1) as h2pool,
            tc.tile_pool(name="b0p", bufs=2) as b0p,
            tc.tile_pool(name="b1p", bufs=2) as b1p,
            tc.tile_pool(name="b2p", bufs=2) as b2p,
            tc.tile_pool(name="psg", bufs=3, space="PSUM") as psg,
            tc.tile_pool(name="pss", bufs=2, space="PSUM") as pss,
        ):
            # HAM warm-up: ~4us of dummy matmuls while weight/x DMAs land,
            # so the PE clock-gate is at 8/8 when real work starts.
            junk = wpool.tile([128, 512], BF16)
            nc.vector.memset(junk[:], 0.0)
            jp = psg.tile([128, 512], F32, tag="gi")
            # just enough PE warm-up to bridge until the first x tile lands
            for _ in range(6):
                nc.tensor.matmul(jp[:], junk[:, 0:128], junk[:])
            # pull the sigmoid/tanh ACT table loads into the DMA fill
            wa = gates.tile([128, 2], BF16, tag="warm")
            nc.scalar.activation(wa[:, 0:1], junk[:, 0:1], AF.Sigmoid)
            nc.scalar.activation(wa[:, 1:2], junk[:, 1:2], AF.Tanh)

            # weights on sync, ahead of the x tiles
            walls = wpool.tile([128, 1568], BF16)
            nc.sync.dma_start(out=walls[:], in_=wall_d.ap(), transpose=True)
            bss = wpool.tile([128, 16], F32)
            nc.gpsimd.dma_start(out=bss[:], in_=bs_d.ap())

            # persistent feature-major h2 strips: col 4+n <-> node n
            h2fB = h2pool.tile([128, PER_CORE + 4], BF16, name="h2fB")
            h2bB = h2pool.tile([128, PER_CORE + 4], BF16, name="h2bB")
            nc.vector.tensor_copy(h2fB[:, 0:4], bss[:, 8:12])
            nc.vector.tensor_copy(h2bB[:, 0:4], bss[:, 12:16])

            # bias column layout in bss: per (layer, dir): z, n
            def bcol(l, d, g):
                i = (l * 2 + d) * 2 + g
                return bss[:, i : i + 1]

            def gru_cell(l, d, rhs_chunks, W, h_out):
                """rhs_chunks: list of [128, W] bf16 APs (K chunks).
                Writes h = sigmoid(-a_z) * tanh(a_n) into h_out."""
                nch = len(rhs_chunks)
                gi = [psg.tile([128, W], F32, tag="gi", name=f"gi{g}")
                      for g in range(2)]
                for g in range(2):
                    for c, rhs in enumerate(rhs_chunks):
                        if l == 0:
                            k = (d * 2 + g) * 128
                            lhsT = walls[:, k : k + 128]
                        else:
                            k = 512 + ((d * 2 + g) * 2 + c) * 128
                            lhsT = walls[:, k : k + 128]
                        for n0 in range(0, W, SUB):
                            nw = min(SUB, W - n0)
                            nc.tensor.matmul(
                                gi[g][:, n0 : n0 + nw], lhsT,
                                rhs[:, n0 : n0 + nw],
                                start=(c == 0), stop=(c == nch - 1),
                            )
                zc = gates.tile([128, W], BF16, tag="zc")
                nc.scalar.activation(zc[:], gi[0][:], AF.Sigmoid,
                                     bias=bcol(l, d, 0))
                ng = gates.tile([128, W], BF16, tag="ng")
                nc.scalar.activation(ng[:], gi[1][:], AF.Tanh,
                                     bias=bcol(l, d, 1))
                nc.vector.tensor_mul(h_out, zc[:], ng[:])

            h1s = {}   # t -> (h1f, h1b)

            def load_x(t):
                xT = xpool.tile([128, TILE], BF16, tag="xT", name=f"xT{t}")
                r0 = t * TILE
                nc.sync.dma_start(out=xT[:], in_=x_d.ap()[r0 : r0 + TILE, :],
                                  transpose=True)
                return xT

            def gru_l1(t, xT):
                h1f = hpool.tile([128, TILE], BF16, tag="h1f")
                gru_cell(0, 0, [xT[:]], TILE, h1f[:])
                h1b = hpool.tile([128, TILE], BF16, tag="h1b")
                gru_cell(0, 1, [xT[:]], TILE, h1b[:])
                h1s[t] = (h1f, h1b)

            def gru_l2(t):
                h1f, h1b = h1s.pop(t)
                c0 = 4 + t * TILE
                gru_cell(1, 0, [h1f[:], h1b[:]], TILE, h2fB[:, c0 : c0 + TILE])
                gru_cell(1, 1, [h1f[:], h1b[:]], TILE, h2bB[:, c0 : c0 + TILE])

            # Stencil strips: strip s projects h2 cols
            # [508 s + 2, +512) -> P[10, 512], stacked 11 strips to a
            # [110, 512] PSUM group (one bank).  Strip s yields out rows
            # [508 s, 508 s + 508) (s=32: 128 rows).
            groups = {}

            def strip_mm(s):
                g, row = s // GRP, (s % GRP) * 32
                if g not in groups:
                    groups[g] = pss.tile([74, SUB], F32, tag="pst",
                                         name=f"pst{g}")
                PS = groups[g]
                w = SUB if s < NSTRIP - 1 else (PER_CORE - STRIDE * s + 4)
                c0 = STRIDE * s
                nc.tensor.matmul(PS[row : row + 10, 0:w], walls[:, 1536:1546],
                                 h2fB[:, c0 : c0 + w], start=True, stop=False)
                nc.tensor.matmul(PS[row : row + 10, 0:w], walls[:, 1546:1556],
                                 h2bB[:, c0 : c0 + w], start=False, stop=True)

            def box_group(g):
                PS = groups.pop(g)
                n = 74
                C = b0p.tile([n, SUB], F32, tag="c")
                nc.vector.tensor_copy(C[:], PS[:])
                B1 = b1p.tile([n, SUB - 2], F32, tag="b1")
                nc.vector.tensor_add(B1[:], C[:, 0 : SUB - 2], C[:, 1 : SUB - 1])
                nc.vector.tensor_add(B1[:], B1[:], C[:, 2:SUB])
                B2 = b2p.tile([n, STRIDE], F32, tag="b2")
                nc.vector.tensor_add(B2[:], B1[:, 0:STRIDE],
                                     B1[:, 1 : STRIDE + 1])
                nc.vector.tensor_add(B2[:], B2[:], B1[:, 2 : STRIDE + 2])
                for k in range(GRP):
                    s = g * GRP + k
                    if s >= NSTRIP:
                        break
                    o0 = STRIDE * s
                    w = min(STRIDE, PER_CORE - o0)
                    eng = nc.gpsimd if s % 2 == 0 else nc.sync
                    eng.dma_start(out=out_d.ap()[:, o0 : o0 + w],
                                  in_=B2[32 * k : 32 * k + 10, 0:w])

            # strips whose h2 window closes with tile t
            ready = {t: [] for t in range(NTILES)}
            for s in range(NSTRIP):
                w = SUB if s < NSTRIP - 1 else (PER_CORE - STRIDE * s + 4)
                end = STRIDE * s + w              # h2buf col end
                t_ready = max(0, (end - 4 - 1) // TILE)
                ready[t_ready].append(s)

            # pipeline: L1(i) | L2(i-1) | strips+box as windows close
            xts = {0: load_x(0), 1: load_x(1)}
            for i in range(0, NTILES + 2):
                if i < NTILES:
                    gru_l1(i, xts.pop(i))
                if i + 2 < NTILES and i + 2 not in xts:
                    xts[i + 2] = load_x(i + 2)
                if 0 <= i - 1 < NTILES:
                    gru_l2(i - 1)
                if 0 <= i - 1 < NTILES:
                    for s in ready[i - 1]:
                        strip_mm(s)
                        if s % GRP == GRP - 1 or s == NSTRIP - 1:
                            box_group(s // GRP)

    nc.compile()
    return nc


def _sigmoid_linfit(U):
    """Per-column least-squares linear fit sigmoid(u) ~ alpha + beta*u."""
    s = 1.0 / (1.0 + np.exp(-U))
    mu = U.mean(axis=0)
    var = U.var(axis=0) + 1e-12
    beta = ((U - mu) * (s - s.mean(axis=0))).mean(axis=0) / var
    alpha = s.mean(axis=0) - beta * mu
    return alpha, beta


def _prep_inputs(inputs):
    bf = ml_dtypes.bfloat16
    x = np.asarray(inputs["x"], np.float32)
    xs = np.ascontiguousarray(x[::8])  # 16384 calibration samples

    def cell_weights(w_ih, b_ih, b_hh, cal_in):
        """Returns (Wz_neg [128,K], Wn_eff [128,K], zbias, nbias)."""
        w_ih = np.asarray(w_ih, np.float32)
        b_ih = np.asarray(b_ih, np.float32)
        b_hh = np.asarray(b_hh, np.float32)
        Wr, Wz, Wn = w_ih[0:128], w_ih[128:256], w_ih[256:384]
        brt = b_ih[0:128] + b_hh[0:128]
        bhn = b_hh[256:384]
        U = cal_in @ Wr.T + brt
        alpha, beta = _sigmoid_linfit(U)
        Wn_eff = Wn + (bhn * beta)[:, None] * Wr
        zbias = -(b_ih[128:256] + b_hh[128:256])
        nbias = b_ih[256:384] + bhn * (alpha + beta * brt)
        return -Wz, Wn_eff, zbias, nbias

    # exact h1 on the calibration sample (for the layer-2 fit)
    h1s = np.concatenate(
        [_gru_np(xs, np.asarray(inputs["w_ih_f1"], np.float32),
                 np.asarray(inputs["b_ih_f1"], np.float32),
                 np.asarray(inputs["b_hh_f1"], np.float32)),
         _gru_np(xs, np.asarray(inputs["w_ih_b1"], np.float32),
                 np.asarray(inputs["b_ih_b1"], np.float32),
                 np.asarray(inputs["b_hh_b1"], np.float32))], axis=1)

    cells = {
        (0, 0): cell_weights(inputs["w_ih_f1"], inputs["b_ih_f1"],
                             inputs["b_hh_f1"], xs),
        (0, 1): cell_weights(inputs["w_ih_b1"], inputs["b_ih_b1"],
                             inputs["b_hh_b1"], xs),
        (1, 0): cell_weights(inputs["w_ih_f2"], inputs["b_ih_f2"],
                             inputs["b_hh_f2"], h1s),
        (1, 1): cell_weights(inputs["w_ih_b2"], inputs["b_ih_b2"],
                             inputs["b_hh_b2"], h1s),
    }

    # w1: [128, (d*2+g)*128]; w2: [128, ((d*2+g)*2+c)*128]; blocks are
    # [in, out] (transposed weight block)
    w1cols, w2cols = [], []
    for d in range(2):
        Wz, Wn, _, _ = cells[(0, d)]
        w1cols.append(Wz.T.copy())
        w1cols.append(Wn.T.copy())
        Wz2, Wn2, _, _ = cells[(1, d)]
        for Wg in (Wz2, Wn2):
            for c in range(2):
                w2cols.append(Wg[:, c * 128 : (c + 1) * 128].T.copy())
    w1 = np.concatenate(w1cols, axis=1).astype(bf)   # [128, 512]
    w2 = np.concatenate(w2cols, axis=1).astype(bf)   # [128, 1024]

    bs = np.zeros((128, 16), np.float32)
    for l in range(2):
        for d in range(2):
            _, _, zb, nb = cells[(l, d)]
            bs[:, (l * 2 + d) * 2 + 0] = zb
            bs[:, (l * 2 + d) * 2 + 1] = nb

    w_g1 = np.asarray(inputs["w_g1"], np.float32)
    w_g2 = np.asarray(inputs["w_g2"], np.float32)
    w_fc = np.asarray(inputs["w_fc"], np.float32)
    W = (w_g1 @ w_g2 @ w_fc) / 9.0  # [256, 10]
    wst = np.concatenate([W[0:128], W[128:256]], axis=1)  # [128, 20]
    wall = np.zeros((1568, 128), np.float32)  # transposed, padded
    wall[0:512] = w1.astype(np.float32).T
    wall[512:1536] = w2.astype(np.float32).T
    wall[1536:1556] = wst.T
    wall = np.ascontiguousarray(wall).astype(bf)

    # exact h2 halo (nodes s-4..s-1) per core, feature-major
    def h2_exact(xrows):
        h1 = np.concatenate(
            [_gru_np(xrows, np.asarray(inputs["w_ih_f1"], np.float32),
                     np.asarray(inputs["b_ih_f1"], np.float32),
                     np.asarray(inputs["b_hh_f1"], np.float32)),
             _gru_np(xrows, np.asarray(inputs["w_ih_b1"], np.float32),
                     np.asarray(inputs["b_ih_b1"], np.float32),
                     np.asarray(inputs["b_hh_b1"], np.float32))], axis=1)
        h2f = _gru_np(h1, np.asarray(inputs["w_ih_f2"], np.float32),
                      np.asarray(inputs["b_ih_f2"], np.float32),
                      np.asarray(inputs["b_hh_f2"], np.float32))
        h2b = _gru_np(h1, np.asarray(inputs["w_ih_b2"], np.float32),
                      np.asarray(inputs["b_ih_b2"], np.float32),
                      np.asarray(inputs["b_hh_b2"], np.float32))
        return np.concatenate([h2f.T, h2b.T], axis=1)  # [128, 8]

    xb = x.astype(bf)
    in_maps = []
    for c in range(NCORES):
        s = c * PER_CORE
        bsc = bs.copy()
        if c > 0:
            bsc[:, 8:16] = h2_exact(x[s - 4 : s])
        in_maps.append({
            "x": np.ascontiguousarray(xb[s : s + PER_CORE]),
            "wall": wall, "bs": bsc,
        })
    return in_maps


def _gru_np(x, w_ih, b_ih, b_hh):
    gi = x @ w_ih.T + b_ih
    ir, iz, inn = gi[:, :128], gi[:, 128:256], gi[:, 256:]
    hr, hz, hn = b_hh[:128], b_hh[128:256], b_hh[256:]
    r = 1.0 / (1.0 + np.exp(-(ir + hr)))
    z = 1.0 / (1.0 + np.exp(-(iz + hz)))
    ng = np.tanh(inn + r * hn)
    return (1.0 - z) * ng


def _fix_boundary(out, inputs, side):
    """Exact (float64) recompute of the 3 boundary rows on one side."""
    M = 16  # margin
    f8 = np.float64
    if side == "left":
        xs = np.asarray(inputs["x"], np.float32)[:M].astype(f8)
    else:
        xs = np.asarray(inputs["x"], np.float32)[-M:].astype(f8)

    def cell(x, tag):
        return _gru_np(x, np.asarray(inputs[f"w_ih_{tag}"], f8),
                       np.asarray(inputs[f"b_ih_{tag}"], f8),
                       np.asarray(inputs[f"b_hh_{tag}"], f8))

    h1 = np.concatenate([cell(xs, "f1"), cell(xs, "b1")], axis=1)
    h2 = np.concatenate([cell(h1, "f2"), cell(h1, "b2")], axis=1)

    c2, c3 = 1.0 / np.sqrt(2.0), 1.0 / np.sqrt(3.0)
    dinv = np.full(M, c3, f8)
    if side == "left":
        dinv[0] = c2
    else:
        dinv[-1] = c2

    def gcn(h, w, b):
        xw = h @ np.asarray(w, f8)
        y = dinv[:, None] * xw
        s = y.copy()
        s[:-1] += y[1:]
        s[1:] += y[:-1]
        return dinv[:, None] * s + np.asarray(b, f8)

    g1 = gcn(h2, inputs["w_g1"], inputs["b_g1"])
    g2 = gcn(g1, inputs["w_g2"], inputs["b_g2"])
    o = g2 @ np.asarray(inputs["w_fc"], f8) + np.asarray(inputs["b_fc"], f8)
    # only the 3 true boundary rows are used; those depend only on
    # in-margin data.
    if side == "left":
        out[0:3] = o[0:3].astype(np.float32)
    else:
        out[-3:] = o[-3:].astype(np.float32)


def kernel(**inputs):
    if "prog" not in _cache:
        _cache["prog"] = _build_program()
    nc = _cache["prog"]

    in_maps = _prep_inputs(inputs)
    res = bass_utils.run_bass_kernel_spmd(nc, in_maps, core_ids=list(range(NCORES)))

    w_g2 = np.asarray(inputs["w_g2"], np.float32)
    w_fc = np.asarray(inputs["w_fc"], np.float32)
    c10 = (np.asarray(inputs["b_g1"], np.float32) @ w_g2 @ w_fc
           + np.asarray(inputs["b_g2"], np.float32) @ w_fc
           + np.asarray(inputs["b_fc"], np.float32))

    out = np.empty((N, 10), np.float32)
    for c in range(NCORES):
        shard = res.results[c]["out"].T + c10  # [16384, 10]
        s = c * PER_CORE
        if c == 0:
            out[0 : PER_CORE - 2] = shard[2:]
        else:
            out[s - 2 : s + PER_CORE - 2] = shard
    _fix_boundary(out, inputs, "left")
    _fix_boundary(out, inputs, "right")
    return out


# revision 24
# speedup vs baseline: 1.1860x; 1.0020x over previous
"""Trainium2 Bass kernel for BiGRU(2-layer) + chain-graph GCN(2) + FC.

Strategy (8 NeuronCores, data-parallel over the node dim):
- seq_len=1, h0=0 => each GRU direction is a per-node gated MLP and the
  r-gate only enters via tanh(a_n + b_hn * sigmoid(a_r)) with b_hn a
  small per-feature constant.  sigmoid(a_r) is replaced by a per-feature
  least-squares linear fit (calibrated on a subsample of the real data),
  which folds the whole r-path into the n-gate weight matrix on the
  host.  Each cell is then: z' = sigmoid(-a_z), n = tanh(a_n~), h=z'*n.
- The two GCN layers + final FC fuse into a [256 -> 10] projection
  followed by a 5-point stencil [1,2,3,2,1]/9 along the node dim.  The
  projection runs on the PE (10-row output), the stencil runs as two
  3-point box-filter passes on the narrow [10, n] data (gpsimd pass 1,
  vector pass 2).  Output is stored feature-major [10, n] and
  transposed on the host.
- Everything runs feature-major ([feat, node] tiles); x is transposed
  on load via DMA-transpose (bf16).  Matmuls are bf16 w/ fp32 PSUM.
- Each core processes 16384 nodes in 16 tiles of 1024, plus a tiny
  prologue supplying the 4-column h2 halo carried between tiles.  The
  stencil stage of tile t is emitted after tile t+1's GRU stage so the
  in-order PE stream has ready matmuls while tile t's gating drains.
  Core c's outputs cover global rows [c*16384-2, c*16384+16382).
- The 3 first / 3 last global rows (graph-boundary degree effects + the
  2 rows no core computes) are recomputed exactly on host in float64.
"""

import numpy as np
import ml_dtypes

import concourse.bacc as bacc
import concourse.mybir as mybir
import concourse.tile as tile
from concourse import bass_utils

N = 131072
NCORES = 8
PER_CORE = N // NCORES          # 16384
TILE = 1024                     # GRU tile width (nodes)
SUB = 512                       # PSUM bank width (fp32 cols)
NTILES = PER_CORE // TILE       # 16

F32 = mybir.dt.float32
BF16 = mybir.dt.bfloat16
AF = mybir.ActivationFunctionType
ALU = mybir.AluOpType

_cache = {}


def _build_program():
    nc = bacc.Bacc("TRN2", target_bir_lowering=False, debug=False)

    x_d = nc.dram_tensor("x", [PER_CORE, 128], BF16, kind="ExternalInput")
    # packed weights: w1 (2 gates x 2 dirs), w2 (x 2 K-chunks), wst
    wall_d = nc.dram_tensor("wall", [128, 1556], BF16, kind="ExternalInput")
    # cols 0:8 = gate biases; cols 8:16 = host-computed exact h2 halo for
    # nodes s-4..s-1 (f cols 8:12, b 12:16)
    bs_d = nc.dram_tensor("bs", [128, 16], F32, kind="ExternalInput")
    out_d = nc.dram_tensor("out", [10, PER_CORE], F32, kind="ExternalOutput")

    STRIDE = 508                     # stencil strip stride (512 - 4)
    NSTRIP = 33                      # ceil(16384 / 508)
    GRP = 3                          # strips per stacked PSUM group
                                     # (matmul PSUM base must be 0/32/64)

    with tile.TileContext(nc) as tc:
        with (
            tc.tile_pool(name="wpool", bufs=1) as wpool,
            tc.tile_pool(name="xpool", bufs=6) as xpool,
            tc.tile_pool(name="gates", bufs=8) as gates,
            tc.tile_pool(name="hpool", bufs=8) as hpool,
            tc.tile_pool(name="h2buf", bufs=1) as h2pool,
            tc.tile_pool(name="b0p", bufs=2) as b0p,
            tc.tile_pool(name="b1p", bufs=2) as b1p,
            tc.tile_pool(name="b2p", bufs=2) as b2p,
            tc.tile_pool(name="psg", bufs=3, space="PSUM") as psg,
            tc.tile_pool(name="pss", bufs=2, space="PSUM") as pss,
        ):
            # HAM warm-up: ~4us of dummy matmuls while weight/x DMAs land,
            # so the PE clock-gate is at 8/8 when real work starts.
            junk = wpool.tile([128, 512], BF16)
            nc.vector.memset(junk[:], 0.0)
            jp = psg.tile([128, 512], F32, tag="gi")
            # just enough PE warm-up to bridge until the first x tile lands
            for _ in range(6):
                nc.tensor.matmul(jp[:], junk[:, 0:128], junk[:])
            # pull the sigmoid/tanh ACT table loads into the DMA fill
            wa = gates.tile([128, 2], BF16, tag="warm")
            nc.scalar.activation(wa[:, 0:1], junk[:, 0:1], AF.Sigmoid)
            nc.scalar.activation(wa[:, 1:2], junk[:, 1:2], AF.Tanh)

            # weights on sync, ahead of the x tiles
            walls = wpool.tile([128, 1556], BF16)
            bss = wpool.tile([128, 16], F32)
            nc.gpsimd.dma_start(out=bss[:], in_=bs_d.ap())

            # persistent feature-major h2 strips: col 4+n <-> node n
            h2fB = h2pool.tile([128, PER_CORE + 4], BF16, name="h2fB")
            h2bB = h2pool.tile([128, PER_CORE + 4], BF16, name="h2bB")
            nc.vector.tensor_copy(h2fB[:, 0:4], bss[:, 8:12])
            nc.vector.tensor_copy(h2bB[:, 0:4], bss[:, 12:16])

            # bias column layout in bss: per (layer, dir): z, n
            def bcol(l, d, g):
                i = (l * 2 + d) * 2 + g
                return bss[:, i : i + 1]

            def gru_cell(l, d, rhs_chunks, W, h_out):
                """rhs_chunks: list of [128, W] bf16 APs (K chunks).
                Writes h = sigmoid(-a_z) * tanh(a_n) into h_out."""
                nch = len(rhs_chunks)
                gi = [psg.tile([128, W], F32, tag="gi", name=f"gi{g}")
                      for g in range(2)]
                for g in range(2):
                    for c, rhs in enumerate(rhs_chunks):
                        if l == 0:
                            k = (d * 2 + g) * 128
                            lhsT = walls[:, k : k + 128]
                        else:
                            k = 512 + ((d * 2 + g) * 2 + c) * 128
                            lhsT = walls[:, k : k + 128]
                        for n0 in range(0, W, SUB):
                            nw = min(SUB, W - n0)
                            nc.tensor.matmul(
                                gi[g][:, n0 : n0 + nw], lhsT,
                                rhs[:, n0 : n0 + nw],
                                start=(c == 0), stop=(c == nch - 1),
                            )
                zc = gates.tile([128, W], BF16, tag="zc")
                nc.scalar.activation(zc[:], gi[0][:], AF.Sigmoid,
                                     bias=bcol(l, d, 0))
                ng = gates.tile([128, W], BF16, tag="ng")
                nc.scalar.activation(ng[:], gi[1][:], AF.Tanh,
                                     bias=bcol(l, d, 1))
                nc.vector.tensor_mul(h_out, zc[:], ng[:])

            h1s = {}   # t -> (h1f, h1b)

            def load_x(t):
                xT = xpool.tile([128, TILE], BF16, tag="xT", name=f"xT{t}")
                r0 = t * TILE
                nc.sync.dma_start(out=xT[:], in_=x_d.ap()[r0 : r0 + TILE, :],
                                  transpose=True)
                return xT

            def gru_l1(t, xT):
                h1f = hpool.tile([128, TILE], BF16, tag="h1f")
                gru_cell(0, 0, [xT[:]], TILE, h1f[:])
                h1b = hpool.tile([128, TILE], BF16, tag="h1b")
                gru_cell(0, 1, [xT[:]], TILE, h1b[:])
                h1s[t] = (h1f, h1b)

            def gru_l2(t):
                h1f, h1b = h1s.pop(t)
                c0 = 4 + t * TILE
                gru_cell(1, 0, [h1f[:], h1b[:]], TILE, h2fB[:, c0 : c0 + TILE])
                gru_cell(1, 1, [h1f[:], h1b[:]], TILE, h2bB[:, c0 : c0 + TILE])

            # Stencil strips: strip s projects h2 cols
            # [508 s + 2, +512) -> P[10, 512], stacked 11 strips to a
            # [110, 512] PSUM group (one bank).  Strip s yields out rows
            # [508 s, 508 s + 508) (s=32: 128 rows).
            groups = {}

            def strip_mm(s):
                g, row = s // GRP, (s % GRP) * 32
                if g not in groups:
                    groups[g] = pss.tile([74, SUB], F32, tag="pst",
                                         name=f"pst{g}")
                PS = groups[g]
                w = SUB if s < NSTRIP - 1 else (PER_CORE - STRIDE * s + 4)
                c0 = STRIDE * s
                nc.tensor.matmul(PS[row : row + 10, 0:w], walls[:, 1536:1546],
                                 h2fB[:, c0 : c0 + w], start=True, stop=False)
                nc.tensor.matmul(PS[row : row + 10, 0:w], walls[:, 1546:1556],
                                 h2bB[:, c0 : c0 + w], start=False, stop=True)

            def box_group(g):
                PS = groups.pop(g)
                n = 74
                C = b0p.tile([n, SUB], F32, tag="c")
                nc.vector.tensor_copy(C[:], PS[:])
                B1 = b1p.tile([n, SUB - 2], F32, tag="b1")
                nc.vector.tensor_add(B1[:], C[:, 0 : SUB - 2], C[:, 1 : SUB - 1])
                nc.vector.tensor_add(B1[:], B1[:], C[:, 2:SUB])
                B2 = b2p.tile([n, STRIDE], F32, tag="b2")
                nc.vector.tensor_add(B2[:], B1[:, 0:STRIDE],
                                     B1[:, 1 : STRIDE + 1])
                nc.vector.tensor_add(B2[:], B2[:], B1[:, 2 : STRIDE + 2])
                for k in range(GRP):
                    s = g * GRP + k
                    if s >= NSTRIP:
                        break
                    o0 = STRIDE * s
                    w = min(STRIDE, PER_CORE - o0)
                    eng = nc.gpsimd if s % 2 == 0 else nc.sync
                    eng.dma_start(out=out_d.ap()[:, o0 : o0 + w],
                                  in_=B2[32 * k : 32 * k + 10, 0:w])

            # strips whose h2 window closes with tile t
            ready = {t: [] for t in range(NTILES)}
            for s in range(NSTRIP):
                w = SUB if s < NSTRIP - 1 else (PER_CORE - STRIDE * s + 4)
                end = STRIDE * s + w              # h2buf col end
                t_ready = max(0, (end - 4 - 1) // TILE)
                ready[t_ready].append(s)

            # pipeline: L1(i) | L2(i-1) | strips+box as windows close
            xts = {0: load_x(0), 1: load_x(1)}
            nc.sync.dma_start(out=walls[:], in_=wall_d.ap())
            for i in range(0, NTILES + 2):
                if i < NTILES:
                    gru_l1(i, xts.pop(i))
                if i + 2 < NTILES and i + 2 not in xts:
                    xts[i + 2] = load_x(i + 2)
                if 0 <= i - 1 < NTILES:
                    gru_l2(i - 1)
                if 0 <= i - 1 < NTILES:
                    for s in ready[i - 1]:
                        strip_mm(s)
                        if s % GRP == GRP - 1 or s == NSTRIP - 1:
                            box_group(s // GRP)

    nc.compile()
    return nc


def _sigmoid_linfit(U):
    """Per-column least-squares linear fit sigmoid(u) ~ alpha + beta*u."""
    s = 1.0 / (1.0 + np.exp(-U))
    mu = U.mean(axis=0)
    var = U.var(axis=0) + 1e-12
    beta = ((U - mu) * (s - s.mean(axis=0))).mean(axis=0) / var
    alpha = s.mean(axis=0) - beta * mu
    return alpha, beta


def _prep_inputs(inputs):
    bf = ml_dtypes.bfloat16
    x = np.asarray(inputs["x"], np.float32)
    xs = np.ascontiguousarray(x[::8])  # 16384 calibration samples

    def cell_weights(w_ih, b_ih, b_hh, cal_in):
        """Returns (Wz_neg [128,K], Wn_eff [128,K], zbias, nbias)."""
        w_ih = np.asarray(w_ih, np.float32)
        b_ih = np.asarray(b_ih, np.float32)
        b_hh = np.asarray(b_hh, np.float32)
        Wr, Wz, Wn = w_ih[0:128], w_ih[128:256], w_ih[256:384]
        brt = b_ih[0:128] + b_hh[0:128]
        bhn = b_hh[256:384]
        U = cal_in @ Wr.T + brt
        alpha, beta = _sigmoid_linfit(U)
        Wn_eff = Wn + (bhn * beta)[:, None] * Wr
        zbias = -(b_ih[128:256] + b_hh[128:256])
        nbias = b_ih[256:384] + bhn * (alpha + beta * brt)
        return -Wz, Wn_eff, zbias, nbias

    # exact h1 on the calibration sample (for the layer-2 fit)
    h1s = np.concatenate(
        [_gru_np(xs, np.asarray(inputs["w_ih_f1"], np.float32),
                 np.asarray(inputs["b_ih_f1"], np.float32),
                 np.asarray(inputs["b_hh_f1"], np.float32)),
         _gru_np(xs, np.asarray(inputs["w_ih_b1"], np.float32),
                 np.asarray(inputs["b_ih_b1"], np.float32),
                 np.asarray(inputs["b_hh_b1"], np.float32))], axis=1)

    cells = {
        (0, 0): cell_weights(inputs["w_ih_f1"], inputs["b_ih_f1"],
                             inputs["b_hh_f1"], xs),
        (0, 1): cell_weights(inputs["w_ih_b1"], inputs["b_ih_b1"],
                             inputs["b_hh_b1"], xs),
        (1, 0): cell_weights(inputs["w_ih_f2"], inputs["b_ih_f2"],
                             inputs["b_hh_f2"], h1s),
        (1, 1): cell_weights(inputs["w_ih_b2"], inputs["b_ih_b2"],
                             inputs["b_hh_b2"], h1s),
    }

    # w1: [128, (d*2+g)*128]; w2: [128, ((d*2+g)*2+c)*128]; blocks are
    # [in, out] (transposed weight block)
    w1cols, w2cols = [], []
    for d in range(2):
        Wz, Wn, _, _ = cells[(0, d)]
        w1cols.append(Wz.T.copy())
        w1cols.append(Wn.T.copy())
        Wz2, Wn2, _, _ = cells[(1, d)]
        for Wg in (Wz2, Wn2):
            for c in range(2):
                w2cols.append(Wg[:, c * 128 : (c + 1) * 128].T.copy())
    w1 = np.concatenate(w1cols, axis=1).astype(bf)   # [128, 512]
    w2 = np.concatenate(w2cols, axis=1).astype(bf)   # [128, 1024]

    bs = np.zeros((128, 16), np.float32)
    for l in range(2):
        for d in range(2):
            _, _, zb, nb = cells[(l, d)]
            bs[:, (l * 2 + d) * 2 + 0] = zb
            bs[:, (l * 2 + d) * 2 + 1] = nb

    w_g1 = np.asarray(inputs["w_g1"], np.float32)
    w_g2 = np.asarray(inputs["w_g2"], np.float32)
    w_fc = np.asarray(inputs["w_fc"], np.float32)
    W = (w_g1 @ w_g2 @ w_fc) / 9.0  # [256, 10]
    wst = np.concatenate([W[0:128], W[128:256]], axis=1)  # [128, 20]
    wall = np.concatenate(
        [w1.astype(np.float32), w2.astype(np.float32), wst],
        axis=1).astype(bf)  # [128, 1556]

    # exact h2 halo (nodes s-4..s-1) per core, feature-major
    def h2_exact(xrows):
        h1 = np.concatenate(
            [_gru_np(xrows, np.asarray(inputs["w_ih_f1"], np.float32),
                     np.asarray(inputs["b_ih_f1"], np.float32),
                     np.asarray(inputs["b_hh_f1"], np.float32)),
             _gru_np(xrows, np.asarray(inputs["w_ih_b1"], np.float32),
                     np.asarray(inputs["b_ih_b1"], np.float32),
                     np.asarray(inputs["b_hh_b1"], np.float32))], axis=1)
        h2f = _gru_np(h1, np.asarray(inputs["w_ih_f2"], np.float32),
                      np.asarray(inputs["b_ih_f2"], np.float32),
                      np.asarray(inputs["b_hh_f2"], np.float32))
        h2b = _gru_np(h1, np.asarray(inputs["w_ih_b2"], np.float32),
                      np.asarray(inputs["b_ih_b2"], np.float32),
                      np.asarray(inputs["b_hh_b2"], np.float32))
        return np.concatenate([h2f.T, h2b.T], axis=1)  # [128, 8]

    xb = x.astype(bf)
    in_maps = []
    for c in range(NCORES):
        s = c * PER_CORE
        bsc = bs.copy()
        if c > 0:
            bsc[:, 8:16] = h2_exact(x[s - 4 : s])
        in_maps.append({
            "x": np.ascontiguousarray(xb[s : s + PER_CORE]),
            "wall": wall, "bs": bsc,
        })
    return in_maps


def _gru_np(x, w_ih, b_ih, b_hh):
    gi = x @ w_ih.T + b_ih
    ir, iz, inn = gi[:, :128], gi[:, 128:256], gi[:, 256:]
    hr, hz, hn = b_hh[:128], b_hh[128:256], b_hh[256:]
    r = 1.0 / (1.0 + np.exp(-(ir + hr)))
    z = 1.0 / (1.0 + np.exp(-(iz + hz)))
    ng = np.tanh(inn + r * hn)
    return (1.0 - z) * ng


def _fix_boundary(out, inputs, side):
    """Exact (float64) recompute of the 3 boundary rows on one side."""
    M = 16  # margin
    f8 = np.float64
    if side == "left":
        xs = np.asarray(inputs["x"], np.float32)[:M].astype(f8)
    else:
        xs = np.asarray(inputs["x"], np.float32)[-M:].astype(f8)

    def cell(x, tag):
        return _gru_np(x, np.asarray(inputs[f"w_ih_{tag}"], f8),
                       np.asarray(inputs[f"b_ih_{tag}"], f8),
                       np.asarray(inputs[f"b_hh_{tag}"], f8))

    h1 = np.concatenate([cell(xs, "f1"), cell(xs, "b1")], axis=1)
    h2 = np.concatenate([cell(h1, "f2"), cell(h1, "b2")], axis=1)

    c2, c3 = 1.0 / np.sqrt(2.0), 1.0 / np.sqrt(3.0)
    dinv = np.full(M, c3, f8)
    if side == "left":
        dinv[0] = c2
    else:
        dinv[-1] = c2

    def gcn(h, w, b):
        xw = h @ np.asarray(w, f8)
        y = dinv[:, None] * xw
        s = y.copy()
        s[:-1] += y[1:]
        s[1:] += y[:-1]
        return dinv[:, None] * s + np.asarray(b, f8)

    g1 = gcn(h2, inputs["w_g1"], inputs["b_g1"])
    g2 = gcn(g1, inputs["w_g2"], inputs["b_g2"])
    o = g2 @ np.asarray(inputs["w_fc"], f8) + np.asarray(inputs["b_fc"], f8)
    # only the 3 true boundary rows are used; those depend only on
    # in-margin data.
    if side == "left":
        out[0:3] = o[0:3].astype(np.float32)
    else:
        out[-3:] = o[-3:].astype(np.float32)


def kernel(**inputs):
    if "prog" not in _cache:
        _cache["prog"] = _build_program()
    nc = _cache["prog"]

    in_maps = _prep_inputs(inputs)
    res = bass_utils.run_bass_kernel_spmd(nc, in_maps, core_ids=list(range(NCORES)))

    w_g2 = np.asarray(inputs["w_g2"], np.float32)
    w_fc = np.asarray(inputs["w_fc"], np.float32)
    c10 = (np.asarray(inputs["b_g1"], np.float32) @ w_g2 @ w_fc
           + np.asarray(inputs["b_g2"], np.float32) @ w_fc
           + np.asarray(inputs["b_fc"], np.float32))

    out = np.empty((N, 10), np.float32)
    for c in range(NCORES):
        shard = res.results[c]["out"].T + c10  # [16384, 10]
        s = c * PER_CORE
        if c == 0:
            out[0 : PER_CORE - 2] = shard[2:]
        else:
            out[s - 2 : s + PER_CORE - 2] = shard
    _fix_boundary(out, inputs, "left")
    _fix_boundary(out, inputs, "right")
    return out
